# revision 16
# baseline (speedup 1.0000x reference)
"""MHSA (RoPE + causal softmax) Trainium2 Bass kernel.

Problem: x[4,2048,1024], Wq/Wk/Wv/Wo[1024,1024] fp32; 16 heads, d_k=64.

Sharding over the 8 NeuronCores: 4-way data-parallel over batch x 2-way
tensor-parallel over heads. core = 2*b + t handles batch b, heads
[t*8, t*8+8). Column-parallel Wq/Wk/Wv, row-parallel Wo; the two TP
partial outputs per batch are summed on the host (the gather step).

Device-side design (per core, all sizes hardcoded):
  - Host supplies x^T [1024,2048] (bf16) so every matmul contraction dim
    (model dim d or k-positions) lands on SBUF partitions. All matmuls
    are bf16 (1 cycle/row, fp32 PSUM accumulation).
  - RoPE: interleaved even/odd pairs are pre-permuted in the Wq/Wk ROWS
    (host side) so each rotation partner lives 16 partitions away within
    a 32-partition quadrant; sin is additionally pre-shuffled on the
    host (sinT2[i] = signed-sin[partner(i)]) so the rotation is
        q' = q * cosT + stream_shuffle(q * sinT2)
    with all DVE intermediates in bf16. Scores are permutation-invariant
    so nothing downstream changes.
  - Attention per head pair (2 heads stacked on 128 partitions, the two
    64-row score matmuls run CONCURRENTLY on PE row-strips h0/h64):
       S^T[j] = K_j @ Q_I^T          (bf16, [128 kpos, <=512 q])
       P^T    = exp(S^T / 8)          (ACT, psum -> bf16 sbuf)
       causal: block-skip j>4I+3, narrow diagonal tiles, one [128,128]
       tri-mask multiply on the diagonal block
       O^T   += [V_j | 1x64]^T @ P^T  (bf16; V carries 64 ones-columns so
                                       rows 64..127 of O^T replicate the
                                       softmax denominator)
       mh^T   = O^T[0:64] * approx_recip(O^T[64:128])
  - Scheduling: the scalar engine (exp) is the co-bottleneck (0.84ns/col
    + 250ns/instr), so attention score tiles are woven at fine grain
    with dense matmul groups that keep the in-order PE queue fed while
    ACT drains:
      warm | proj0 + attn0-scores | proj1 + attn0-AV | attn1 + proj2 |
      attn2 + proj3 | attn3 + outproj(0,1,2) | outproj(3)
  - Output projection back in [s, d] layout (lhsT = mh^T tiles) so the
    DRAM write is contiguous; host sums the two TP partials per batch.
"""
import numpy as np
import ml_dtypes

import concourse.bass as bass
from concourse import bacc
import concourse.tile as tile
import concourse.mybir as mybir
from concourse.bass_utils import run_bass_kernel_spmd

B, S, D = 4, 2048, 1024
HEADS, DK = 16, 64
THETA = 10000.0
TP, DP = 2, 4
HL = HEADS // TP            # 8 local heads per core
DL = HL * DK                # 512 local projection width
P = 128
SB = 512                    # q super-tile width
NSB = S // SB               # 4 q super-tiles (I)
NST = S // P                # 16 k-tiles (j)
NDC = D // P                # 8 contraction chunks over model dim
NPAIR = HL // 2             # 4 local head pairs

f32 = mybir.dt.float32
bf16 = mybir.dt.bfloat16
SWAP16 = [(i + 16) % 32 for i in range(32)]


def _build():
    ALU = mybir.AluOpType
    FX = mybir.ActivationFunctionType
    nc = bacc.Bacc(None, target_bir_lowering=False)

    xT = nc.dram_tensor("xT", [D, S], bf16, kind="ExternalInput")
    wqkvT = nc.dram_tensor("wqkvT", [D, 3 * DL], bf16, kind="ExternalInput")
    woT = nc.dram_tensor("woT", [DL, D], bf16, kind="ExternalInput")
    cosf = nc.dram_tensor("cosf", [DK, S], bf16, kind="ExternalInput")
    sinf = nc.dram_tensor("sinf", [DK, S], bf16, kind="ExternalInput")
    maskt = nc.dram_tensor("maskt", [P, P], bf16, kind="ExternalInput")
    out = nc.dram_tensor("out", [S, D], f32, kind="ExternalOutput")

    xT_t = xT.rearrange("(dc p) s -> p dc s", p=P)        # [128, 8, 2048]
    wq_t = wqkvT.rearrange("(dc p) f -> p dc f", p=P)     # [128, 8, 1536]
    wo_t = woT.rearrange("(c p) f -> p c f", p=P)         # [128, 4, 1024]

    with tile.TileContext(nc) as tc:
        with (
            tc.tile_pool(name="wpool", bufs=1) as wpool,
            tc.tile_pool(name="kpool", bufs=1) as kpool,
            tc.tile_pool(name="xpool", bufs=2) as xpool,
            tc.tile_pool(name="qpool", bufs=3) as qpool,
            tc.tile_pool(name="tpool", bufs=2) as tpool,
            tc.tile_pool(name="ptpool", bufs=4) as ptpool,
            tc.tile_pool(name="mpool", bufs=4) as mpool,
            tc.tile_pool(name="spool", bufs=2) as spool,
            tc.tile_pool(name="opool", bufs=2) as opool,
            tc.tile_pool(name="ps_proj", bufs=2, space="PSUM") as ps_proj,
            tc.tile_pool(name="ps_s", bufs=2, space="PSUM") as ps_s,
            tc.tile_pool(name="ps_av", bufs=1, space="PSUM") as ps_av,
        ):
            # --- head DMAs: one batched issue per tensor, spread over the
            # three independent queue rings (sync HWDGE / scalar HWDGE /
            # gpsimd SWDGE). scalar gets only x0, issued before any exp.
            w_sb = wpool.tile([P, NDC, 3 * DL], bf16)
            xts = [xpool.tile([P, NDC, SB], bf16, tag="xt", name=f"xt{i}")
                   for i in range(2)]
            cos_sb = wpool.tile([P, S], bf16)
            sin_sb = wpool.tile([P, S], bf16)
            mask_sb = wpool.tile([P, P], bf16)
            wo_sb = wpool.tile([P, DL // P, D], bf16)

            # Two priority-ordered rings (per-ring FIFO): weights chained on
            # sync; constants first then chunked x0 on scalar, so the first
            # Q-projection group paces on per-dc arrivals while RoPE
            # constants are already resident. Nothing else competes early.
            nc.scalar.dma_start(cos_sb[0:DK, :], cosf[:])
            nc.scalar.dma_start(sin_sb[0:DK, :], sinf[:])
            nc.scalar.dma_start(mask_sb[:], maskt[:])
            nc.scalar.dma_start(cos_sb[DK:P, :], cos_sb[0:DK, :])
            nc.scalar.dma_start(sin_sb[DK:P, :], sin_sb[0:DK, :])
            for dc in range(NDC):
                nc.sync.dma_start(w_sb[:, dc, 0:DL], wq_t[:, dc, 0:DL])
                nc.scalar.dma_start(xts[0][:, dc, :], xT_t[:, dc, 0:SB])
            nc.sync.dma_start(w_sb[:, :, DL : 2 * DL], wq_t[:, :, DL : 2 * DL])
            nc.sync.dma_start(w_sb[:, :, 2 * DL : 3 * DL],
                              wq_t[:, :, 2 * DL : 3 * DL])
            nc.scalar.dma_start(xts[1][:, :, :], xT_t[:, :, SB : 2 * SB])
            nc.sync.dma_start(wo_sb[:, :, :], wo_t[:, :, :])

            ktall = kpool.tile([P, NPAIR, S], bf16)
            v_sb = kpool.tile([P, NST, HL, 2 * DK], bf16)
            nc.vector.memset(v_sb[:, :, :, DK : 2 * DK], 1.0)

            # PE warm-up: hold the HAM clock at 2.4GHz through the
            # DMA-bound head.
            warm = wpool.tile([P, SB], bf16)
            nc.vector.memset(warm[:], 0.0)
            warm_sink = wpool.tile([P, SB], f32)
            pw = ps_s.tile([P, 2, SB], f32, tag="pss", name="warm")
            for r in range(12):
                nc.tensor.matmul(pw[:, 0, :], warm[:, 0:P], warm[:],
                                 start=True, stop=True)
            nc.vector.tensor_copy(warm_sink[:], pw[:, 0, :])

            def emit_qk_group(I, fc, xt, qt):
                scol = slice(I * SB, (I + 1) * SB)
                pp = ps_proj.tile([P, SB], f32, tag="pp", name=f"pp{I}_{fc}")
                for dc in range(NDC):
                    nc.tensor.matmul(
                        pp[:],
                        w_sb[:, dc, fc * P : (fc + 1) * P],
                        xt[:, dc, :],
                        start=(dc == 0),
                        stop=(dc == NDC - 1),
                    )
                dest = qt[:, fc, :] if fc < NPAIR else ktall[:, fc - NPAIR, scol]
                nc.vector.tensor_tensor(dest, pp[:], cos_sb[:, scol], ALU.mult)
                tsb = tpool.tile([P, SB], bf16, tag="tsb", name=f"tsb{I}_{fc}")
                nc.vector.tensor_tensor(tsb[:], pp[:], sin_sb[:, scol], ALU.mult)
                tsh = tpool.tile([P, SB], bf16, tag="tsh", name=f"tsh{I}_{fc}")
                nc.vector.stream_shuffle(tsh[:], tsb[:], mask=SWAP16)
                nc.gpsimd.tensor_tensor(dest, dest, tsh[:], ALU.add)

            def emit_v_group(I, st, xt):
                pp = ps_proj.tile([P, SB], f32, tag="pp", name=f"ppv{I}_{st}")
                for dc in range(NDC):
                    nc.tensor.matmul(
                        pp[:],
                        xt[:, dc, st * P : (st + 1) * P],
                        w_sb[:, dc, 2 * DL : 3 * DL],
                        start=(dc == 0),
                        stop=(dc == NDC - 1),
                    )
                nc.vector.tensor_copy(
                    v_sb[:, I * 4 + st, :, 0:DK],
                    pp[:].rearrange("p (h d) -> p h d", h=HL),
                )

            def emit_scores(I, c, qt, fills=()):
                # scores + exp for all j-tiles of pair c; `fills` maps
                # j -> list of callbacks emitted right after tile j to
                # keep the PE queue dense while ACT drains.
                njt = 4 * I + 4
                pts = []
                for j in range(njt):
                    m = j - 4 * I
                    off = m * P if m > 0 else 0
                    N = SB - off
                    pss = ps_s.tile([P, 2, SB], f32, tag="pss",
                                    name=f"pss{I}_{c}_{j}")
                    for half in (0, 1):
                        pr = 64 * half
                        nc.tensor.matmul(
                            pss[:, half, :N],
                            ktall[pr : pr + 64, c, j * P : (j + 1) * P],
                            qt[pr : pr + 64, c, off:SB],
                            start=True,
                            stop=True,
                        )
                    pt = ptpool.tile(
                        [P, 2, SB], bf16, tag="pt", name=f"pt{I}_{c}_{j}",
                        bufs=NST + 2,
                    )
                    nc.scalar.activation(
                        pt[:, :, :N], pss[:, :, :N], FX.Exp, scale=0.125
                    )
                    if m >= 0:
                        nc.vector.tensor_tensor(
                            pt[:, :, 0:P], pt[:, :, 0:P],
                            mask_sb[:, None, :].to_broadcast((P, 2, P)),
                            ALU.mult,
                        )
                    pts.append((pt, off, N))
                    for f in fills.get(j, ()) if isinstance(fills, dict) else ():
                        f()
                return pts

            def emit_av(I, c, pts, mhI):
                njt = 4 * I + 4
                po0 = ps_av.tile([P, SB], f32, tag="po0", name=f"po{I}_{c}_0",
                                 bufs=1)
                po1 = ps_av.tile([P, SB], f32, tag="po1", name=f"po{I}_{c}_1",
                                 bufs=1)
                for j in range(njt):
                    pt, off, N = pts[j]
                    for half, po in ((0, po0), (1, po1)):
                        nc.tensor.matmul(
                            po[:, off:SB],
                            v_sb[:, j, 2 * c + half, :],
                            pt[:, half, :N],
                            start=(j == 0),
                            stop=(j == njt - 1),
                            skip_group_check=True,
                        )
                for half, po in ((0, po0), (1, po1)):
                    pr = 64 * half
                    lsb = spool.tile([64, SB], f32, tag="lsb",
                                     name=f"lsb{I}_{c}_{half}")
                    nc.vector.tensor_copy(lsb[:], po[DK:P, :])
                    rec = spool.tile([64, SB], f32, tag="rec",
                                     name=f"rec{I}_{c}_{half}")
                    nc.vector.reciprocal_approx_fast(rec[:], lsb[:])
                    nc.vector.tensor_tensor(
                        mhI[pr : pr + 64, c, :], po[0:DK, :], rec[:], ALU.mult
                    )

            def emit_outproj(Io, mh, sts):
                for st in sts:
                    osb = opool.tile([P, D], f32, tag="osb",
                                     name=f"osb{Io}_{st}")
                    for oh in (0, 1):
                        pq = ps_proj.tile([P, SB], f32, tag="pp",
                                          name=f"pq{Io}_{st}_{oh}")
                        for c2 in range(NPAIR):
                            nc.tensor.matmul(
                                pq[:],
                                mh[:, c2, st * P : (st + 1) * P],
                                wo_sb[:, c2, oh * SB : (oh + 1) * SB],
                                start=(c2 == 0),
                                stop=(c2 == NPAIR - 1),
                            )
                        nc.vector.tensor_copy(osb[:, oh * SB : (oh + 1) * SB],
                                              pq[:])
                        nc.sync.dma_start(
                            out[(Io * 4 + st) * P : (Io * 4 + st + 1) * P,
                                oh * SB : (oh + 1) * SB],
                            osb[:, oh * SB : (oh + 1) * SB],
                        )

            qts = [qpool.tile([P, NPAIR, SB], bf16, tag="qt", name=f"qt{i}")
                   for i in range(NSB)]
            mhs = [mpool.tile([P, NPAIR, SB], bf16, tag="mh", name=f"mh{i}")
                   for i in range(NSB)]

            # --- proj0 with attn0 scores woven in (ACT is idle here) ---
            pts0 = {}
            for c in range(NPAIR):
                emit_qk_group(0, c, xts[0], qts[0])
            for c in range(NPAIR):
                emit_qk_group(0, NPAIR + c, xts[0], qts[0])
                pts0[c] = emit_scores(0, c, qts[0])
            for st in range(4):
                emit_v_group(0, st, xts[0])

            # --- proj1 with attn0 AV woven in ---
            xt2 = xpool.tile([P, NDC, SB], bf16, tag="xt", name="xt2")
            nc.sync.dma_start(xt2[:, :, :], xT_t[:, :, 2 * SB : 3 * SB])
            for c in range(NPAIR):
                emit_qk_group(1, c, xts[1], qts[1])
                emit_qk_group(1, NPAIR + c, xts[1], qts[1])
                emit_av(0, c, pts0[c], mhs[0])
            for st in range(4):
                emit_v_group(1, st, xts[1])

            # --- attn1 with proj2 fills ---
            xt3 = xpool.tile([P, NDC, SB], bf16, tag="xt", name="xt3")
            nc.sync.dma_start(xt3[:, :, :], xT_t[:, :, 3 * SB : 4 * SB])
            for c in range(NPAIR):
                fills = {
                    2: [lambda c=c: emit_qk_group(2, c, xt2, qts[2])],
                    5: [lambda c=c: emit_qk_group(2, NPAIR + c, xt2, qts[2])],
                    7: [lambda c=c: emit_v_group(2, c, xt2)],
                }
                pts = emit_scores(1, c, qts[1], fills)
                emit_av(1, c, pts, mhs[1])

            # --- attn2 with proj3 fills ---
            for c in range(NPAIR):
                fills = {
                    3: [lambda c=c: emit_qk_group(3, c, xt3, qts[3])],
                    7: [lambda c=c: emit_qk_group(3, NPAIR + c, xt3, qts[3])],
                    11: [lambda c=c: emit_v_group(3, c, xt3)],
                }
                pts = emit_scores(2, c, qts[2], fills)
                emit_av(2, c, pts, mhs[2])

            # --- attn3 with outproj(0,1,2) fills ---
            for c in range(NPAIR):
                fills = {
                    4: [lambda c=c: emit_outproj(0, mhs[0], sts=(c,))],
                    9: [lambda c=c: emit_outproj(1, mhs[1], sts=(c,))],
                    13: [lambda c=c: emit_outproj(2, mhs[2], sts=(c,))],
                }
                pts = emit_scores(3, c, qts[3], fills)
                emit_av(3, c, pts, mhs[3])

            # --- tail ---
            emit_outproj(3, mhs[3], sts=range(4))
    nc.finalize()
    return nc


_NC = None


def _get_nc():
    global _NC
    if _NC is None:
        _NC = _build()
    return _NC


def _host_prep(Wq, Wk, Wv, Wo):
    t = np.arange(DK // 2)
    qd, rd = t // 16, t % 16
    perm = np.empty(DK, np.int64)
    perm[qd * 32 + rd] = 2 * t
    perm[qd * 32 + 16 + rd] = 2 * t + 1

    Wq_p = Wq.reshape(HEADS, DK, D)[:, perm, :].reshape(HEADS * DK, D)
    Wk_p = Wk.reshape(HEADS, DK, D)[:, perm, :].reshape(HEADS * DK, D)

    pos = np.arange(S, dtype=np.float64)
    inv = 1.0 / THETA ** (np.arange(0, DK, 2).astype(np.float64) / DK)  # [32]
    ang = inv[:, None] * pos[None, :]                                   # [32, S]
    cos32 = np.cos(ang).astype(np.float32)
    sin32 = np.sin(ang).astype(np.float32)
    cosf = np.empty((DK, S), np.float32)
    sinf = np.empty((DK, S), np.float32)
    rows_lo = qd * 32 + rd
    rows_hi = qd * 32 + 16 + rd
    cosf[rows_lo] = cos32[t]
    cosf[rows_hi] = cos32[t]
    # sin is pre-shuffled for q' = q*cos + shuffle(q*sinT2):
    # sinT2[row] = signed-sin[partner(row)]
    sinf[rows_lo] = sin32[t]
    sinf[rows_hi] = -sin32[t]

    mask01 = (
        np.arange(P)[:, None] <= np.arange(P)[None, :]
    ).astype(ml_dtypes.bfloat16)

    per_tp = []
    for tp in range(TP):
        sl = slice(tp * DL, (tp + 1) * DL)
        wqkvT = np.ascontiguousarray(
            np.concatenate([Wq_p[sl], Wk_p[sl], Wv[sl]], axis=0).T
        ).astype(ml_dtypes.bfloat16)
        woT = np.ascontiguousarray(Wo[:, sl].T).astype(ml_dtypes.bfloat16)
        per_tp.append((wqkvT, woT))
    return per_tp, cosf.astype(ml_dtypes.bfloat16), sinf.astype(ml_dtypes.bfloat16), mask01


def kernel(x, Wq, Wk, Wv, Wo):
    x = np.asarray(x, np.float32)
    Wq = np.asarray(Wq, np.float32)
    Wk = np.asarray(Wk, np.float32)
    Wv = np.asarray(Wv, np.float32)
    Wo = np.asarray(Wo, np.float32)

    per_tp, cosf, sinf, mask01 = _host_prep(Wq, Wk, Wv, Wo)
    xTs = [np.ascontiguousarray(x[b].T).astype(ml_dtypes.bfloat16) for b in range(B)]

    in_maps = []
    for core in range(DP * TP):
        b, tp = core // TP, core % TP
        wqkvT, woT = per_tp[tp]
        in_maps.append(
            {
                "xT": xTs[b],
                "wqkvT": wqkvT,
                "woT": woT,
                "cosf": cosf,
                "sinf": sinf,
                "maskt": mask01,
            }
        )

    nc = _get_nc()
    res = run_bass_kernel_spmd(nc, in_maps, core_ids=list(range(DP * TP)))
    out = np.empty((B, S, D), np.float32)
    for b in range(B):
        out[b] = res.results[b * TP]["out"] + res.results[b * TP + 1]["out"]
    return out


# revision 17
# speedup vs baseline: 1.1912x; 1.1912x over previous
"""MHSA (RoPE + causal softmax) Trainium2 Bass kernel.

Problem: x[4,2048,1024], Wq/Wk/Wv/Wo[1024,1024] fp32; 16 heads, d_k=64.

Sharding over the 8 NeuronCores: 4-way data-parallel over batch x 2-way
tensor-parallel over heads. core = 2*b + t handles batch b, heads
[t*8, t*8+8). Column-parallel Wq/Wk/Wv, row-parallel Wo; the two TP
partial outputs per batch are summed on the host (the gather step).

Device-side design (per core, all sizes hardcoded):
  - Host supplies x^T [1024,2048] (bf16) so every matmul contraction dim
    (model dim d or k-positions) lands on SBUF partitions. All matmuls
    are bf16 (1 cycle/row, fp32 PSUM accumulation).
  - RoPE: interleaved even/odd pairs are pre-permuted in the Wq/Wk ROWS
    (host side) so each rotation partner lives 16 partitions away within
    a 32-partition quadrant; sin is additionally pre-shuffled on the
    host (sinT2[i] = signed-sin[partner(i)]) so the rotation is
        q' = q * cosT + stream_shuffle(q * sinT2)
    with all DVE intermediates in bf16. Scores are permutation-invariant
    so nothing downstream changes.
  - Attention per head pair (2 heads stacked on 128 partitions, the two
    64-row score matmuls run CONCURRENTLY on PE row-strips h0/h64):
       S^T[j] = K_j @ Q_I^T          (bf16, [128 kpos, <=512 q])
       P^T    = exp(S^T / 8)          (ACT, psum -> bf16 sbuf)
       causal: block-skip j>4I+3, narrow diagonal tiles, one [128,128]
       tri-mask multiply on the diagonal block
       O^T   += [V_j | 1x64]^T @ P^T  (bf16; V carries 64 ones-columns so
                                       rows 64..127 of O^T replicate the
                                       softmax denominator)
       mh^T   = O^T[0:64] * approx_recip(O^T[64:128])
  - Scheduling: the scalar engine (exp) is the co-bottleneck (0.84ns/col
    + 250ns/instr), so attention score tiles are woven at fine grain
    with dense matmul groups that keep the in-order PE queue fed while
    ACT drains:
      warm | proj0 + attn0-scores | proj1 + attn0-AV | attn1 + proj2 |
      attn2 + proj3 | attn3 + outproj(0,1,2) | outproj(3)
  - Output projection back in [s, d] layout (lhsT = mh^T tiles) so the
    DRAM write is contiguous; host sums the two TP partials per batch.
"""
import numpy as np
import ml_dtypes

import concourse.bass as bass
from concourse import bacc
import concourse.tile as tile
import concourse.mybir as mybir
from concourse.bass_utils import run_bass_kernel_spmd

B, S, D = 4, 2048, 1024
HEADS, DK = 16, 64
THETA = 10000.0
TP, DP = 2, 4
HL = HEADS // TP            # 8 local heads per core
DL = HL * DK                # 512 local projection width
P = 128
SB = 512                    # q super-tile width
NSB = S // SB               # 4 q super-tiles (I)
NST = S // P                # 16 k-tiles (j)
NDC = D // P                # 8 contraction chunks over model dim
NPAIR = HL // 2             # 4 local head pairs

f32 = mybir.dt.float32
bf16 = mybir.dt.bfloat16
SWAP16 = [(i + 16) % 32 for i in range(32)]


def _build():
    ALU = mybir.AluOpType
    FX = mybir.ActivationFunctionType
    nc = bacc.Bacc(None, target_bir_lowering=False)

    xT = nc.dram_tensor("xT", [D, S], bf16, kind="ExternalInput")
    wqkvT = nc.dram_tensor("wqkvT", [D, 3 * DL], bf16, kind="ExternalInput")
    woT = nc.dram_tensor("woT", [DL, D], bf16, kind="ExternalInput")
    cosf = nc.dram_tensor("cosf", [DK, S], bf16, kind="ExternalInput")
    sinf = nc.dram_tensor("sinf", [DK, S], bf16, kind="ExternalInput")
    maskt = nc.dram_tensor("maskt", [P, P], bf16, kind="ExternalInput")
    out = nc.dram_tensor("out", [S, D], f32, kind="ExternalOutput")

    xT_t = xT.rearrange("(dc p) s -> p dc s", p=P)        # [128, 8, 2048]
    wq_t = wqkvT.rearrange("(dc p) f -> p dc f", p=P)     # [128, 8, 1536]
    wo_t = woT.rearrange("(c p) f -> p c f", p=P)         # [128, 4, 1024]

    with tile.TileContext(nc) as tc:
        with (
            tc.tile_pool(name="wpool", bufs=1) as wpool,
            tc.tile_pool(name="kpool", bufs=1) as kpool,
            tc.tile_pool(name="xpool", bufs=2) as xpool,
            tc.tile_pool(name="qpool", bufs=3) as qpool,
            tc.tile_pool(name="tpool", bufs=2) as tpool,
            tc.tile_pool(name="ptpool", bufs=4) as ptpool,
            tc.tile_pool(name="mpool", bufs=4) as mpool,
            tc.tile_pool(name="spool", bufs=2) as spool,
            tc.tile_pool(name="opool", bufs=2) as opool,
            tc.tile_pool(name="ps_proj", bufs=2, space="PSUM") as ps_proj,
            tc.tile_pool(name="ps_s", bufs=2, space="PSUM") as ps_s,
            tc.tile_pool(name="ps_av", bufs=1, space="PSUM") as ps_av,
        ):
            # --- head DMAs: one batched issue per tensor, spread over the
            # three independent queue rings (sync HWDGE / scalar HWDGE /
            # gpsimd SWDGE). scalar gets only x0, issued before any exp.
            w_sb = wpool.tile([P, NDC, 3 * DL], bf16)
            xts = [xpool.tile([P, NDC, SB], bf16, tag="xt", name=f"xt{i}")
                   for i in range(2)]
            cos_sb = wpool.tile([P, S], bf16)
            sin_sb = wpool.tile([P, S], bf16)
            mask_sb = wpool.tile([P, P], bf16)
            wo_sb = wpool.tile([P, DL // P, D], bf16)

            nc.sync.dma_start(w_sb[:, :, 0:DL], wq_t[:, :, 0:DL])
            nc.scalar.dma_start(xts[0][:, :, :], xT_t[:, :, 0:SB])
            nc.gpsimd.dma_start(cos_sb[0:DK, :], cosf[:])
            nc.gpsimd.dma_start(sin_sb[0:DK, :], sinf[:])
            nc.gpsimd.dma_start(mask_sb[:], maskt[:])
            nc.gpsimd.dma_start(cos_sb[DK:P, :], cos_sb[0:DK, :])
            nc.gpsimd.dma_start(sin_sb[DK:P, :], sin_sb[0:DK, :])
            nc.gpsimd.dma_start(w_sb[:, :, DL : 2 * DL], wq_t[:, :, DL : 2 * DL])
            nc.sync.dma_start(w_sb[:, :, 2 * DL : 3 * DL],
                              wq_t[:, :, 2 * DL : 3 * DL])
            nc.scalar.dma_start(xts[1][:, :, :], xT_t[:, :, SB : 2 * SB])
            nc.gpsimd.dma_start(wo_sb[:, :, :], wo_t[:, :, :])

            ktall = kpool.tile([P, NPAIR, S], bf16)
            v_sb = kpool.tile([P, NST, HL, 2 * DK], bf16)
            nc.vector.memset(v_sb[:, :, :, DK : 2 * DK], 1.0)

            # PE warm-up: hold the HAM clock at 2.4GHz through the
            # DMA-bound head.
            warm = wpool.tile([P, SB], bf16)
            nc.vector.memset(warm[:], 0.0)
            warm_sink = wpool.tile([P, SB], f32)
            pw = ps_s.tile([P, 2, SB], f32, tag="pss", name="warm")
            for r in range(22):
                nc.tensor.matmul(pw[:, 0, :], warm[:, 0:P], warm[:],
                                 start=True, stop=True)
            nc.vector.tensor_copy(warm_sink[:], pw[:, 0, :])

            def emit_qk_group(I, fc, xt, qt):
                scol = slice(I * SB, (I + 1) * SB)
                pp = ps_proj.tile([P, SB], f32, tag="pp", name=f"pp{I}_{fc}")
                for dc in range(NDC):
                    nc.tensor.matmul(
                        pp[:],
                        w_sb[:, dc, fc * P : (fc + 1) * P],
                        xt[:, dc, :],
                        start=(dc == 0),
                        stop=(dc == NDC - 1),
                    )
                dest = qt[:, fc, :] if fc < NPAIR else ktall[:, fc - NPAIR, scol]
                nc.vector.tensor_tensor(dest, pp[:], cos_sb[:, scol], ALU.mult)
                tsb = tpool.tile([P, SB], bf16, tag="tsb", name=f"tsb{I}_{fc}")
                nc.vector.tensor_tensor(tsb[:], pp[:], sin_sb[:, scol], ALU.mult)
                tsh = tpool.tile([P, SB], bf16, tag="tsh", name=f"tsh{I}_{fc}")
                nc.vector.stream_shuffle(tsh[:], tsb[:], mask=SWAP16)
                nc.gpsimd.tensor_tensor(dest, dest, tsh[:], ALU.add)

            def emit_v_group(I, st, xt):
                pp = ps_proj.tile([P, SB], f32, tag="pp", name=f"ppv{I}_{st}")
                for dc in range(NDC):
                    nc.tensor.matmul(
                        pp[:],
                        xt[:, dc, st * P : (st + 1) * P],
                        w_sb[:, dc, 2 * DL : 3 * DL],
                        start=(dc == 0),
                        stop=(dc == NDC - 1),
                    )
                nc.vector.tensor_copy(
                    v_sb[:, I * 4 + st, :, 0:DK],
                    pp[:].rearrange("p (h d) -> p h d", h=HL),
                )

            def emit_scores(I, c, qt, fills=()):
                # scores + exp for all j-tiles of pair c; `fills` maps
                # j -> list of callbacks emitted right after tile j to
                # keep the PE queue dense while ACT drains.
                njt = 4 * I + 4
                pts = []
                for j in range(njt):
                    m = j - 4 * I
                    off = m * P if m > 0 else 0
                    N = SB - off
                    pss = ps_s.tile([P, 2, SB], f32, tag="pss",
                                    name=f"pss{I}_{c}_{j}")
                    for half in (0, 1):
                        pr = 64 * half
                        nc.tensor.matmul(
                            pss[:, half, :N],
                            ktall[pr : pr + 64, c, j * P : (j + 1) * P],
                            qt[pr : pr + 64, c, off:SB],
                            start=True,
                            stop=True,
                        )
                    pt = ptpool.tile(
                        [P, 2, SB], bf16, tag="pt", name=f"pt{I}_{c}_{j}",
                        bufs=NST + 2,
                    )
                    nc.scalar.activation(
                        pt[:, :, :N], pss[:, :, :N], FX.Exp, scale=0.125
                    )
                    if m >= 0:
                        nc.vector.tensor_tensor(
                            pt[:, :, 0:P], pt[:, :, 0:P],
                            mask_sb[:, None, :].to_broadcast((P, 2, P)),
                            ALU.mult,
                        )
                    pts.append((pt, off, N))
                    for f in fills.get(j, ()) if isinstance(fills, dict) else ():
                        f()
                return pts

            def emit_av(I, c, pts, mhI):
                njt = 4 * I + 4
                po0 = ps_av.tile([P, SB], f32, tag="po0", name=f"po{I}_{c}_0",
                                 bufs=1)
                po1 = ps_av.tile([P, SB], f32, tag="po1", name=f"po{I}_{c}_1",
                                 bufs=1)
                for j in range(njt):
                    pt, off, N = pts[j]
                    for half, po in ((0, po0), (1, po1)):
                        nc.tensor.matmul(
                            po[:, off:SB],
                            v_sb[:, j, 2 * c + half, :],
                            pt[:, half, :N],
                            start=(j == 0),
                            stop=(j == njt - 1),
                            skip_group_check=True,
                        )
                for half, po in ((0, po0), (1, po1)):
                    pr = 64 * half
                    lsb = spool.tile([64, SB], f32, tag="lsb",
                                     name=f"lsb{I}_{c}_{half}")
                    nc.vector.tensor_copy(lsb[:], po[DK:P, :])
                    rec = spool.tile([64, SB], f32, tag="rec",
                                     name=f"rec{I}_{c}_{half}")
                    nc.vector.reciprocal_approx_fast(rec[:], lsb[:])
                    nc.vector.tensor_tensor(
                        mhI[pr : pr + 64, c, :], po[0:DK, :], rec[:], ALU.mult
                    )

            def emit_outproj(Io, mh, sts):
                for st in sts:
                    osb = opool.tile([P, D], f32, tag="osb",
                                     name=f"osb{Io}_{st}")
                    for oh in (0, 1):
                        pq = ps_proj.tile([P, SB], f32, tag="pp",
                                          name=f"pq{Io}_{st}_{oh}")
                        for c2 in range(NPAIR):
                            nc.tensor.matmul(
                                pq[:],
                                mh[:, c2, st * P : (st + 1) * P],
                                wo_sb[:, c2, oh * SB : (oh + 1) * SB],
                                start=(c2 == 0),
                                stop=(c2 == NPAIR - 1),
                            )
                        nc.vector.tensor_copy(osb[:, oh * SB : (oh + 1) * SB],
                                              pq[:])
                        nc.sync.dma_start(
                            out[(Io * 4 + st) * P : (Io * 4 + st + 1) * P,
                                oh * SB : (oh + 1) * SB],
                            osb[:, oh * SB : (oh + 1) * SB],
                        )

            qts = [qpool.tile([P, NPAIR, SB], bf16, tag="qt", name=f"qt{i}")
                   for i in range(NSB)]
            mhs = [mpool.tile([P, NPAIR, SB], bf16, tag="mh", name=f"mh{i}")
                   for i in range(NSB)]

            # --- proj0 with attn0 scores woven in (ACT is idle here) ---
            pts0 = {}
            for c in range(NPAIR):
                emit_qk_group(0, c, xts[0], qts[0])
            for c in range(NPAIR):
                emit_qk_group(0, NPAIR + c, xts[0], qts[0])
                pts0[c] = emit_scores(0, c, qts[0])
            for st in range(4):
                emit_v_group(0, st, xts[0])

            # --- proj1 with attn0 AV woven in ---
            xt2 = xpool.tile([P, NDC, SB], bf16, tag="xt", name="xt2")
            nc.sync.dma_start(xt2[:, :, :], xT_t[:, :, 2 * SB : 3 * SB])
            for c in range(NPAIR):
                emit_qk_group(1, c, xts[1], qts[1])
                emit_qk_group(1, NPAIR + c, xts[1], qts[1])
                emit_av(0, c, pts0[c], mhs[0])
            for st in range(4):
                emit_v_group(1, st, xts[1])

            # --- attn1 with proj2 fills ---
            xt3 = xpool.tile([P, NDC, SB], bf16, tag="xt", name="xt3")
            nc.sync.dma_start(xt3[:, :, :], xT_t[:, :, 3 * SB : 4 * SB])
            for c in range(NPAIR):
                fills = {
                    2: [lambda c=c: emit_qk_group(2, c, xt2, qts[2])],
                    5: [lambda c=c: emit_qk_group(2, NPAIR + c, xt2, qts[2])],
                    7: [lambda c=c: emit_v_group(2, c, xt2)],
                }
                pts = emit_scores(1, c, qts[1], fills)
                emit_av(1, c, pts, mhs[1])

            # --- attn2 with proj3 fills ---
            for c in range(NPAIR):
                fills = {
                    3: [lambda c=c: emit_qk_group(3, c, xt3, qts[3])],
                    7: [lambda c=c: emit_qk_group(3, NPAIR + c, xt3, qts[3])],
                    11: [lambda c=c: emit_v_group(3, c, xt3)],
                }
                pts = emit_scores(2, c, qts[2], fills)
                emit_av(2, c, pts, mhs[2])

            # --- attn3 with outproj(0,1,2) fills ---
            for c in range(NPAIR):
                fills = {
                    4: [lambda c=c: emit_outproj(0, mhs[0], sts=(c,))],
                    9: [lambda c=c: emit_outproj(1, mhs[1], sts=(c,))],
                    13: [lambda c=c: emit_outproj(2, mhs[2], sts=(c,))],
                }
                pts = emit_scores(3, c, qts[3], fills)
                emit_av(3, c, pts, mhs[3])

            # --- tail ---
            emit_outproj(3, mhs[3], sts=range(4))
    nc.finalize()
    return nc


_NC = None


def _get_nc():
    global _NC
    if _NC is None:
        _NC = _build()
    return _NC


def _host_prep(Wq, Wk, Wv, Wo):
    t = np.arange(DK // 2)
    qd, rd = t // 16, t % 16
    perm = np.empty(DK, np.int64)
    perm[qd * 32 + rd] = 2 * t
    perm[qd * 32 + 16 + rd] = 2 * t + 1

    Wq_p = Wq.reshape(HEADS, DK, D)[:, perm, :].reshape(HEADS * DK, D)
    Wk_p = Wk.reshape(HEADS, DK, D)[:, perm, :].reshape(HEADS * DK, D)

    pos = np.arange(S, dtype=np.float64)
    inv = 1.0 / THETA ** (np.arange(0, DK, 2).astype(np.float64) / DK)  # [32]
    ang = inv[:, None] * pos[None, :]                                   # [32, S]
    cos32 = np.cos(ang).astype(np.float32)
    sin32 = np.sin(ang).astype(np.float32)
    cosf = np.empty((DK, S), np.float32)
    sinf = np.empty((DK, S), np.float32)
    rows_lo = qd * 32 + rd
    rows_hi = qd * 32 + 16 + rd
    cosf[rows_lo] = cos32[t]
    cosf[rows_hi] = cos32[t]
    # sin is pre-shuffled for q' = q*cos + shuffle(q*sinT2):
    # sinT2[row] = signed-sin[partner(row)]
    sinf[rows_lo] = sin32[t]
    sinf[rows_hi] = -sin32[t]

    mask01 = (
        np.arange(P)[:, None] <= np.arange(P)[None, :]
    ).astype(ml_dtypes.bfloat16)

    per_tp = []
    for tp in range(TP):
        sl = slice(tp * DL, (tp + 1) * DL)
        wqkvT = np.ascontiguousarray(
            np.concatenate([Wq_p[sl], Wk_p[sl], Wv[sl]], axis=0).T
        ).astype(ml_dtypes.bfloat16)
        woT = np.ascontiguousarray(Wo[:, sl].T).astype(ml_dtypes.bfloat16)
        per_tp.append((wqkvT, woT))
    return per_tp, cosf.astype(ml_dtypes.bfloat16), sinf.astype(ml_dtypes.bfloat16), mask01


def kernel(x, Wq, Wk, Wv, Wo):
    x = np.asarray(x, np.float32)
    Wq = np.asarray(Wq, np.float32)
    Wk = np.asarray(Wk, np.float32)
    Wv = np.asarray(Wv, np.float32)
    Wo = np.asarray(Wo, np.float32)

    per_tp, cosf, sinf, mask01 = _host_prep(Wq, Wk, Wv, Wo)
    xTs = [np.ascontiguousarray(x[b].T).astype(ml_dtypes.bfloat16) for b in range(B)]

    in_maps = []
    for core in range(DP * TP):
        b, tp = core // TP, core % TP
        wqkvT, woT = per_tp[tp]
        in_maps.append(
            {
                "xT": xTs[b],
                "wqkvT": wqkvT,
                "woT": woT,
                "cosf": cosf,
                "sinf": sinf,
                "maskt": mask01,
            }
        )

    nc = _get_nc()
    res = run_bass_kernel_spmd(nc, in_maps, core_ids=list(range(DP * TP)))
    out = np.empty((B, S, D), np.float32)
    for b in range(B):
        out[b] = res.results[b * TP]["out"] + res.results[b * TP + 1]["out"]
    return out
